# revision 5
# baseline (speedup 1.0000x reference)
"""Hybrid Trainium2 kernel for batched GNN message passing.

Two concurrent gather paths split the 500k edges per their bottleneck engine
(measured on HW: near-pure ap-side APF=0.999 is fastest — concurrent dma_gather
traffic slows the ap side more than it helps; the dma path now only absorbs
per-cell quota overflow):

  * dma-side (SDMA engines): the baseline HBM dma_gather path — 2KB f32 rows,
    per-slot psum segment sums in input-channel space, projected at the end.
    Rate wall: ~55ns/row (SDMA per-descriptor overhead).
  * ap-side (GpSimd Q7 cores): x is re-laid-out host-side as f16 slab^T
    tensors ([128, SLAB_N, 4]; channel 2*inc+b at (p=c//4, n, k=c%4)).
    Slabs stream into SBUF (10 rounds); gpsimd.ap_gather selects edge rows
    channel-major at ~32ns/row; PE stage 1 projects each edge through W
    (which simultaneously flips the layout edge-major); PE stage 2 does the
    per-slot segment sum in OUTPUT space, accumulated in SBUF across rounds.

  The dma-side per-slot epilogue (transpose + project) merges the ap-side
  accumulator and bias at the very end.
"""

import math
import os as _os

import numpy as np

# ---------------------------------------------------------------- problem dims
B = 2
N_IN = 50000
N_OUT = 12500
NNZ = 500000
IN_C = 256
OUT_C = 256
N_CORES = 8
PB = 128
H16 = 32768  # int16 index limit for HBM dma_gather
C2 = 2 * IN_C  # interleaved row width

RND = int(_os.environ.get("K_RND", "10"))  # slab rounds
SLAB_N = N_IN // RND
APF = float(_os.environ.get("K_APF", "0.999"))  # fraction of edges on ap path
TQ = 32  # ap tile size quantum
APCALL = 1024  # max idxs per ap_gather call
GCH = int(_os.environ.get("K_GCH", "4"))  # 128-edge tiles per dma_gather call
NSWQ = int(_os.environ.get("K_NSWQ", "4"))

_CACHE = {}
LAST_RESULTS = None


# ---------------------------------------------------------------- host planning
def _bin_rows(rows, cols):
    """Pack output rows into (core, slot) bins of <=128 rows balancing total
    edge count. Returns NB, rowsets[core][slot] = sorted row ids."""
    Tr = np.bincount(rows, minlength=N_OUT)
    NB = -(-N_OUT // (PB * N_CORES))
    nbins = N_CORES * NB
    t = max(Tr.sum() / nbins, 1.0)

    order = np.argsort(-Tr, kind="stable")
    binT = np.zeros(nbins)
    binN = np.zeros(nbins, dtype=np.int64)
    bin_rows = [[] for _ in range(nbins)]
    for r in order:
        score = (binT + Tr[r]) / t
        score[binN >= PB] = np.inf
        b = int(score.argmin())
        binT[b] += Tr[r]
        binN[b] += 1
        bin_rows[b].append(int(r))

    q = np.argsort(-binT)
    rowsets = [[None] * NB for _ in range(N_CORES)]
    for s in range(NB):
        grp = q[s * N_CORES : (s + 1) * N_CORES]
        for c, b in enumerate(grp):
            rowsets[c][s] = np.array(sorted(bin_rows[b]), dtype=np.int64)
    return NB, rowsets


def _split_cells(rows, cols, NB, rowsets):
    """Per (core, slot): edge ids + rloc; split into ap cells (slot, round)
    and dma low/high remainder. Returns per-core dicts + uniform shape info."""
    order_r = np.argsort(rows, kind="stable")
    bnd_r = np.searchsorted(rows[order_r], np.arange(N_OUT + 1))

    percore = []
    cell_n = np.zeros((N_CORES, RND, NB), dtype=np.int64)
    dmaL_n = np.zeros((N_CORES, NB), dtype=np.int64)
    dmaH_n = np.zeros((N_CORES, NB), dtype=np.int64)
    core_edges = []
    for c in range(N_CORES):
        slots_e = []
        for s in range(NB):
            rowlist = rowsets[c][s]
            per_row = [order_r[bnd_r[r] : bnd_r[r + 1]] for r in rowlist]
            eids = (np.concatenate(per_row) if per_row else
                    np.empty(0, np.int64))
            rloc = np.repeat(np.arange(len(rowlist)),
                             [len(e) for e in per_row])
            co = np.argsort(cols[eids], kind="stable")
            eids, rloc = eids[co], rloc[co]
            g = cols[eids] // SLAB_N
            cells = []
            for r in range(RND):
                m = g == r
                cells.append((eids[m], rloc[m]))
                cell_n[c, r, s] = int(m.sum())
            slots_e.append(cells)
        core_edges.append(slots_e)
    # common ap quota per (round, slot): APF of the mean cell, TQ-quantized
    Tq = (np.round(APF * cell_n.mean(axis=0) / TQ) * TQ).astype(np.int64)

    for c in range(N_CORES):
        slots = []
        for s in range(NB):
            ap_cells = []
            dma_lo_e, dma_lo_r = [], []
            dma_hi_e, dma_hi_r = [], []
            for r in range(RND):
                ce, cr = core_edges[c][s][r]
                k = min(int(Tq[r, s]), len(ce))
                ap_cells.append((ce[:k], cr[:k]))
                rest_e, rest_r = ce[k:], cr[k:]
                ml = cols[rest_e] < H16
                dma_lo_e.append(rest_e[ml])
                dma_lo_r.append(rest_r[ml])
                dma_hi_e.append(rest_e[~ml])
                dma_hi_r.append(rest_r[~ml])
            lo = np.concatenate(dma_lo_e) if dma_lo_e else np.empty(0, np.int64)
            lor = np.concatenate(dma_lo_r) if dma_lo_r else np.empty(0, np.int64)
            hi = np.concatenate(dma_hi_e) if dma_hi_e else np.empty(0, np.int64)
            hir = np.concatenate(dma_hi_r) if dma_hi_r else np.empty(0, np.int64)
            ol = np.argsort(cols[lo], kind="stable")
            oh = np.argsort(cols[hi], kind="stable")
            dmaL_n[c, s] = len(lo)
            dmaH_n[c, s] = len(hi)
            slots.append({
                "ap": ap_cells,
                "lo": (lo[ol], lor[ol]),
                "hi": (hi[oh], hir[oh]),
            })
        percore.append(slots)

    # uniform ap cell sizes: the common quota; >= TQ so every slot is init'd
    TAP = Tq.copy()  # [RND, NB]
    for s in range(NB):
        if TAP[:, s].sum() == 0:
            TAP[0, s] = TQ
    TPBL = (-(-dmaL_n.max(axis=0) // PB)).astype(np.int64)  # [NB]
    TPBH = (-(-dmaH_n.max(axis=0) // PB)).astype(np.int64)
    for s in range(NB):
        if TPBL[s] + TPBH[s] == 0:
            TPBL[s] = 1
    return percore, TAP, TPBL, TPBH


def _ap_schedule(TAP):
    """Tile/call schedule shared by all cores.
    Returns: calls = list of (round, start_q, n_idx) with start_q in 16-idx
    units within the full wrapped stream; tiles = list of dicts."""
    calls = []
    tiles = []
    pos = 0  # global idx position across all rounds
    for r in range(RND):
        # cells of this round, slot-major
        cell_sizes = [int(TAP[r, s]) for s in range(TAP.shape[1])]
        # build tiles for the round
        rtiles = []
        for s, n in enumerate(cell_sizes):
            off = 0
            while off < n:
                e = min(PB, n - off)
                rtiles.append({"slot": s, "E": e,
                               "first": off == 0, "last": off + e >= n})
                off += e
        # group tiles into calls <= APCALL
        i = 0
        while i < len(rtiles):
            nidx = 0
            j = i
            while j < len(rtiles) and nidx + rtiles[j]["E"] <= APCALL:
                nidx += rtiles[j]["E"]
                j += 1
            call_id = len(calls)
            off_in_call = 0
            for t in rtiles[i:j]:
                t["call"] = call_id
                t["off"] = off_in_call
                off_in_call += t["E"]
            calls.append((r, pos // 16, nidx))
            pos += nidx
            i = j
        tiles.extend(rtiles)
    assert pos % 16 == 0
    # tile column index in rowsap/valsap
    for j, t in enumerate(tiles):
        t["col"] = j
    return calls, tiles, pos


def _wrap16(flat):
    n = len(flat)
    assert n % 16 == 0
    w = np.ascontiguousarray(flat.reshape(n // 16, 16).T.astype(np.int16))
    return np.ascontiguousarray(np.tile(w, (8, 1)))


def _pack_core(c, percore, TAP, TPBL, TPBH, tiles, total_ap, rows, cols, vals,
               bias, rowsets):
    NB = TAP.shape[1]
    slots = percore[c]

    # ---- ap streams
    idx_ap = np.zeros(total_ap, dtype=np.int64)
    ntap = len(tiles)
    rows_ap = np.zeros((PB, ntap), dtype=np.float32)
    vals_ap = np.zeros((PB, ntap), dtype=np.float32)
    pos = 0
    tix = 0
    for r in range(RND):
        for s in range(NB):
            n = int(TAP[r, s])
            if n == 0:
                continue
            ce, cr = slots[s]["ap"][r]
            k = len(ce)
            idx_ap[pos : pos + k] = cols[ce] - r * SLAB_N
            # per-tile rloc/vals columns
            off = 0
            while off < n:
                e = min(PB, n - off)
                lo = off
                hi = min(off + e, k)
                if hi > lo:
                    m = hi - lo
                    rows_ap[:m, tix] = cr[lo:hi]
                    vals_ap[:m, tix] = vals[ce[lo:hi]]
                tix += 1
                off += e
            pos += n
    assert pos == total_ap and tix == ntap

    # ---- dma streams (baseline format)
    NL, NH = int(TPBL.sum()), int(TPBH.sum())
    NTD = NL + NH
    rloc_flat = np.zeros(NTD * PB, dtype=np.float32)
    vals_flat = np.zeros(NTD * PB, dtype=np.float32)
    lowE, highE = [], []
    bias_c = np.zeros((NB * PB, OUT_C), dtype=np.float32)
    dpos = 0
    for s in range(NB):
        rowlist = rowsets[c][s]
        if len(rowlist):
            bias_c[s * PB : s * PB + len(rowlist)] = bias[rowlist]
        for (lst, rl), tpb, base, acc in (
            (slots[s]["lo"], int(TPBL[s]), 0, lowE),
            (slots[s]["hi"], int(TPBH[s]), H16, highE),
        ):
            kq = tpb * PB
            if kq == 0:
                assert len(lst) == 0
                continue
            ne = len(lst)
            assert ne <= kq, (ne, kq)
            rloc_flat[dpos : dpos + ne] = rl
            vals_flat[dpos : dpos + ne] = vals[lst]
            cc = np.zeros(kq, dtype=np.int64)
            cc[:ne] = cols[lst] - base
            acc.append(cc)
            dpos += kq
    assert dpos == NTD * PB

    rowsT = np.ascontiguousarray(rloc_flat.reshape(NTD, PB).T)
    valsT = np.ascontiguousarray(vals_flat.reshape(NTD, PB).T)
    idxLW = _wrap16(np.concatenate(lowE) if lowE else np.empty(0, np.int64))
    idxHW = _wrap16(np.concatenate(highE) if highE else np.empty(0, np.int64))
    if idxLW.size == 0:
        idxLW = np.zeros((PB, 8), np.int16)
    if idxHW.size == 0:
        idxHW = np.zeros((PB, 8), np.int16)
    return {
        "idxAP": _wrap16(idx_ap),
        "rowsAP": np.ascontiguousarray(rows_ap),
        "valsAP": np.ascontiguousarray(vals_ap),
        "rowsT": rowsT,
        "valsT": valsT,
        "idxLW": idxLW,
        "idxHW": idxHW,
        "biasC": bias_c,
    }


def _host_arrays(x, weight):
    # dma-side xi: batch-major interleave [n, inc + 256*b] f32
    xi = np.ascontiguousarray(np.concatenate([x[0], x[1]], axis=1))
    # ap-side slabs: batch-minor interleave c=2*inc+b, [128, SLAB_N, 4] f16
    xi_il = np.empty((N_IN, C2), dtype=np.float16)
    xi_il[:, 0::2] = x[0]
    xi_il[:, 1::2] = x[1]
    slabs = np.ascontiguousarray(
        xi_il.reshape(RND, SLAB_N, PB, 4).transpose(0, 2, 1, 3)
        .reshape(RND, PB, SLAB_N * 4)
    )
    w16 = weight.astype(np.float16)
    # stage-1 rhs: [W_even | W_odd], W_even[p,:] = W[2p,:]
    w4 = np.ascontiguousarray(
        np.concatenate([w16[0::2, :], w16[1::2, :]], axis=1)
    )
    # epilogue proj rhs: [W[0:128] | W[128:256]]
    wT = np.ascontiguousarray(
        np.concatenate([w16[0:PB, :], w16[PB:2 * PB, :]], axis=1)
    )
    iota2d = np.ascontiguousarray(
        np.tile(np.arange(PB, dtype=np.float32), (PB, 1))
    )
    ident16 = np.eye(PB, dtype=np.float16)
    return xi, slabs, w4, wT, iota2d, ident16


# ---------------------------------------------------------------- device build
def _build(NB, TAP, TPBL, TPBH, ap_calls, ap_tiles, total_ap, reps=1):
    import concourse.bacc as bacc
    import concourse.mybir as mybir
    from concourse.tile import TileContext

    f32 = mybir.dt.float32
    f32r = mybir.dt.float32r
    f16 = mybir.dt.float16
    i16 = mybir.dt.int16

    NL, NH = int(TPBL.sum()), int(TPBH.sum())
    NTD = NL + NH
    NTAP = len(ap_tiles)

    nc = bacc.Bacc("TRN2", target_bir_lowering=False, debug=False,
                   num_devices=N_CORES, num_swdge_queues=NSWQ)

    xi_d = nc.dram_tensor("xi", [N_IN, C2], f32r, kind="ExternalInput")
    slab_d = nc.dram_tensor("slabs", [RND, PB, SLAB_N * 4], f16,
                            kind="ExternalInput")
    idxap_d = nc.dram_tensor("idxAP", [PB, max(total_ap // 16, 8)], i16,
                             kind="ExternalInput")
    rowsap_d = nc.dram_tensor("rowsAP", [PB, NTAP], f32, kind="ExternalInput")
    valsap_d = nc.dram_tensor("valsAP", [PB, NTAP], f32, kind="ExternalInput")
    idxl_d = nc.dram_tensor("idxLW", [PB, max(NL * 8, 8)], i16,
                            kind="ExternalInput")
    idxh_d = nc.dram_tensor("idxHW", [PB, max(NH * 8, 8)], i16,
                            kind="ExternalInput")
    rows_d = nc.dram_tensor("rowsT", [PB, NTD], f32, kind="ExternalInput")
    vals_d = nc.dram_tensor("valsT", [PB, NTD], f32, kind="ExternalInput")
    bias_d = nc.dram_tensor("biasC", [NB * PB, OUT_C], f32,
                            kind="ExternalInput")
    w4_d = nc.dram_tensor("w4", [PB, 2 * OUT_C], f16, kind="ExternalInput")
    wt_d = nc.dram_tensor("wT", [PB, 2 * OUT_C], f16, kind="ExternalInput")
    iota_d = nc.dram_tensor("iota", [PB, PB], f32, kind="ExternalInput")
    ident_d = nc.dram_tensor("ident16", [PB, PB], f16, kind="ExternalInput")
    out_d = nc.dram_tensor("out", [B, NB * PB, OUT_C], f32,
                           kind="ExternalOutput")

    # dma-side schedule: slot-major; slot s emitted during round rof(s)
    dma_sched = []  # per slot: list of (first, last, stream, streamtile)
    nl = nh = 0
    for s in range(NB):
        ent = []
        tpb = int(TPBL[s] + TPBH[s])
        t = 0
        for _ in range(int(TPBL[s])):
            ent.append((t == 0, t == tpb - 1, "l", nl))
            nl += 1
            t += 1
        for _ in range(int(TPBH[s])):
            ent.append((t == 0, t == tpb - 1, "h", nh))
            nh += 1
            t += 1
        dma_sched.append(ent)
    slots_of_round = [[] for _ in range(RND)]
    for s in range(NB):
        slots_of_round[min(s * RND // NB, RND - 1)].append(s)

    # first nonempty ap cell per slot (for accumulator init)
    first_cell = {}
    for t in ap_tiles:
        key = t["slot"]
        if key not in first_cell and t["first"]:
            first_cell[key] = t["col"]

    ap_tiles_by_call = {}
    for t in ap_tiles:
        ap_tiles_by_call.setdefault(t["call"], []).append(t)
    ap_calls_by_round = {}
    for ci, (r, sq, nidx) in enumerate(ap_calls):
        ap_calls_by_round.setdefault(r, []).append((ci, sq, nidx))

    def body(nc, tc, pools):
        (cpool, slabpool, gappool, xwpool, glpool, ghpool, sdpool, sapool,
         opool, bpool, trpool, papool, xwp_pool, st2pool, ptpool, popool) = pools
        iota_sb = cpool.tile([PB, PB], f32, tag="iota")
        ident_sb = cpool.tile([PB, PB], f16, tag="ident")
        w4_sb = cpool.tile([PB, 2 * OUT_C], f16, tag="w4")
        wt_sb = cpool.tile([PB, 2 * OUT_C], f16, tag="wt")
        idxap_sb = cpool.tile([PB, max(total_ap // 16, 8)], i16, tag="idxap")
        rowsap_sb = cpool.tile([PB, NTAP], f32, tag="rowsap")
        valsap_sb = cpool.tile([PB, NTAP], f32, tag="valsap")
        idxl_sb = cpool.tile([PB, max(NL * 8, 8)], i16, tag="idxl")
        idxh_sb = cpool.tile([PB, max(NH * 8, 8)], i16, tag="idxh")
        rows_sb = cpool.tile([PB, NTD], f32, tag="rows")
        vals_sb = cpool.tile([PB, NTD], f32, tag="vals")
        for sb, d in ((iota_sb, iota_d), (ident_sb, ident_d), (w4_sb, w4_d),
                      (wt_sb, wt_d), (idxap_sb, idxap_d), (rowsap_sb, rowsap_d),
                      (valsap_sb, valsap_d), (idxl_sb, idxl_d),
                      (idxh_sb, idxh_d), (rows_sb, rows_d), (vals_sb, vals_d)):
            nc.sync.dma_start(out=sb[:], in_=d[:])

        acc = [cpool.tile([PB, C2], f32, tag=f"acc{s}", name=f"acc{s}")
               for s in range(NB)]
        seg = [cpool.tile([PB, C2], f16, tag=f"seg{s}", name=f"seg{s}")
               for s in range(NB)]

        qctr = [0]

        def dma_gather_call(pool, tag, idx_sb, src_ap, t0, nstream):
            n = min(GCH, nstream - t0)
            g = pool.tile([PB, GCH * C2], f32r, tag=tag)
            nc.gpsimd.dma_gather(
                out_ap=g[:, : n * C2].rearrange("p (t e) -> p t e", e=C2),
                in_ap=src_ap,
                idxs_ap=idx_sb[:, t0 * 8 : (t0 + n) * 8],
                num_idxs=n * PB,
                num_idxs_reg=n * PB,
                elem_size=C2,
                queue_num=qctr[0] % NSWQ,
            )
            qctr[0] += 1
            return g

        # ---------------- per-round emission
        # dma-side state
        dma_state = {"gl": None, "gh": None, "pacc": None}

        def emit_dma_tile(s, ent):
            first, last, stream, st = ent
            if stream == "l":
                if st % GCH == 0:
                    dma_state["gl"] = dma_gather_call(
                        glpool, "gl", idxl_sb, xi_d[:], st, NL)
                g, off = dma_state["gl"], st % GCH
            else:
                if st % GCH == 0:
                    dma_state["gh"] = dma_gather_call(
                        ghpool, "gh", idxh_sb, xi_d[H16:, :], st, NH)
                g, off = dma_state["gh"], st % GCH
            j = dma_tile_index[s] + dma_tile_base[s]
            if first:
                dma_state["pacc"] = papool.tile([PB, C2], f32, name="pacc")
            s_t = sdpool.tile([PB, PB], f32r, tag="sd")
            nc.vector.tensor_scalar(
                out=s_t[:], in0=iota_sb[:],
                scalar1=rows_sb[:, j : j + 1],
                scalar2=vals_sb[:, j : j + 1],
                op0=mybir.AluOpType.is_equal,
                op1=mybir.AluOpType.mult,
            )
            nc.tensor.matmul(
                out=dma_state["pacc"][:],
                lhsT=s_t[:],
                rhs=g[:, off * C2 : (off + 1) * C2],
                start=first, stop=last,
            )
            dma_tile_index[s] += 1
            if last:
                nc.scalar.copy(out=seg[s][:], in_=dma_state["pacc"][:])

        dma_tile_base = np.concatenate([[0], np.cumsum(
            (TPBL + TPBH).astype(np.int64))])[:NB]
        dma_tile_index = [0] * NB

        st2_state = {}

        def emit_ap_call(ci, sq, nidx, slab_sb):
            g = gappool.tile([PB, APCALL * 4], f16, tag="gap")
            nc.gpsimd.ap_gather(
                out_ap=g[:, : nidx * 4].rearrange("p (i d) -> p i d", d=4),
                in_ap=slab_sb[:].rearrange("p (n d) -> p n d", d=4),
                idxs_ap=idxap_sb[:, sq : sq + nidx // 16],
                channels=PB,
                num_elems=SLAB_N,
                d=4,
                num_idxs=nidx,
            )
            gr = g[:, : nidx * 4].rearrange("p (i d) -> p i d", d=4)
            for t in ap_tiles_by_call.get(ci, []):
                E, o, s = t["E"], t["off"], t["slot"]
                xwp = xwp_pool.tile([PB, C2], f32)
                for k in (0, 2, 1, 3):
                    hb = (k % 2) * OUT_C
                    nc.tensor.matmul(
                        out=xwp[:E, hb : hb + OUT_C],
                        lhsT=gr[:, o : o + E, k],
                        rhs=w4_sb[:, (k // 2) * OUT_C : (k // 2 + 1) * OUT_C],
                        start=(k < 2), stop=(k >= 2),
                    )
                xw_sb = xwpool.tile([PB, C2], f16, tag="xw")
                nc.scalar.copy(out=xw_sb[:E, :], in_=xwp[:E, :])
                s_t = sapool.tile([PB, PB], f16, tag="sa")
                nc.vector.tensor_scalar(
                    out=s_t[:E, :], in0=iota_sb[:E, :],
                    scalar1=rowsap_sb[:E, t["col"] : t["col"] + 1],
                    scalar2=valsap_sb[:E, t["col"] : t["col"] + 1],
                    op0=mybir.AluOpType.is_equal,
                    op1=mybir.AluOpType.mult,
                )
                if t["first"]:
                    st2_state[s] = st2pool.tile([PB, C2], f32, name="st2")
                nc.tensor.matmul(
                    out=st2_state[s][:],
                    lhsT=s_t[:E, :],
                    rhs=xw_sb[:E, :],
                    start=t["first"], stop=t["last"],
                )
                if t["last"]:
                    if s not in acc_inited:
                        nc.scalar.copy(out=acc[s][:], in_=st2_state[s][:])
                        acc_inited.add(s)
                    else:
                        nc.vector.tensor_tensor(
                            out=acc[s][:], in0=acc[s][:],
                            in1=st2_state[s][:],
                            op=mybir.AluOpType.add,
                        )

        acc_inited = set()

        for r in range(RND):
            slab_sb = slabpool.tile([PB, SLAB_N * 4], f16, tag="slab")
            nc.sync.dma_start(out=slab_sb[:], in_=slab_d[r])
            rcalls = ap_calls_by_round.get(r, [])
            rslots = slots_of_round[r]
            # interleave ap calls with this round's dma slots
            dma_queue = []
            for s in rslots:
                for ent in dma_sched[s]:
                    dma_queue.append((s, ent))
            di = 0
            n_ap = len(rcalls)
            # dma tiles to emit between consecutive ap calls
            per = -(-len(dma_queue) // max(n_ap, 1))
            for ai in range(max(n_ap, 1)):
                if ai < n_ap:
                    ci, sq, nidx = rcalls[ai]
                    emit_ap_call(ci, sq, nidx, slab_sb)
                for _ in range(per):
                    if di < len(dma_queue):
                        s, ent = dma_queue[di]
                        emit_dma_tile(s, ent)
                        di += 1
            while di < len(dma_queue):
                s, ent = dma_queue[di]
                emit_dma_tile(s, ent)
                di += 1

        # ---------------- epilogue: project dma-side seg, merge ap acc + bias
        for s in range(NB):
            bias_sb = bpool.tile([PB, OUT_C], f32, tag="bias")
            nc.sync.dma_start(out=bias_sb[:],
                              in_=bias_d[s * PB : (s + 1) * PB, :])
            for b in range(B):
                trs = []
                for kk in range(2):
                    ptr = ptpool.tile([PB, PB], f16)
                    nc.tensor.transpose(
                        out=ptr[:],
                        in_=seg[s][:, b * IN_C + kk * PB :
                                   b * IN_C + (kk + 1) * PB],
                        identity=ident_sb[:],
                    )
                    trk = trpool.tile([PB, PB], f16, tag="tr")
                    nc.scalar.copy(out=trk[:], in_=ptr[:])
                    trs.append(trk)
                po = popool.tile([PB, OUT_C], f32)
                for kk in range(2):
                    nc.tensor.matmul(
                        out=po[:],
                        lhsT=trs[kk][:],
                        rhs=wt_sb[:, kk * OUT_C : (kk + 1) * OUT_C],
                        start=(kk == 0), stop=(kk == 1),
                    )
                tmp = opool.tile([PB, OUT_C], f32, tag="tmp")
                nc.vector.tensor_tensor(
                    out=tmp[:], in0=acc[s][:, b * OUT_C : (b + 1) * OUT_C],
                    in1=bias_sb[:], op=mybir.AluOpType.add,
                )
                osb = opool.tile([PB, OUT_C], f32, tag="o")
                nc.vector.tensor_tensor(
                    out=osb[:], in0=po[:], in1=tmp[:],
                    op=mybir.AluOpType.add,
                )
                nc.sync.dma_start(
                    out=out_d[b, s * PB : (s + 1) * PB, :], in_=osb[:],
                )

    with TileContext(nc) as tc:
        with (
            tc.tile_pool(name="const", bufs=1) as cpool,
            tc.tile_pool(name="slab", bufs=2) as slabpool,
            tc.tile_pool(name="gap", bufs=2) as gappool,
            tc.tile_pool(name="xw", bufs=3) as xwpool,
            tc.tile_pool(name="gl", bufs=2) as glpool,
            tc.tile_pool(name="gh", bufs=2) as ghpool,
            tc.tile_pool(name="sd", bufs=4) as sdpool,
            tc.tile_pool(name="sa", bufs=4) as sapool,
            tc.tile_pool(name="o", bufs=4) as opool,
            tc.tile_pool(name="bias", bufs=2) as bpool,
            tc.tile_pool(name="tr", bufs=4) as trpool,
            tc.tile_pool(name="pacc", bufs=2, space="PSUM") as papool,
            tc.tile_pool(name="xwp", bufs=2, space="PSUM") as xwp_pool,
            tc.tile_pool(name="st2", bufs=2, space="PSUM") as st2pool,
            tc.tile_pool(name="ptr", bufs=1, space="PSUM") as ptpool,
            tc.tile_pool(name="pout", bufs=1, space="PSUM") as popool,
        ):
            pools = (cpool, slabpool, gappool, xwpool, glpool, ghpool, sdpool,
                     sapool, opool, bpool, trpool, papool, xwp_pool, st2pool,
                     ptpool, popool)
            if reps == 1:
                body(nc, tc, pools)
            else:
                with tc.For_i(0, reps, 1):
                    body(nc, tc, pools)

    nc.compile()
    return nc


# ---------------------------------------------------------------- entry points
def _prepare(inputs):
    rows = np.asarray(inputs["rows"], dtype=np.int64)
    cols = np.asarray(inputs["cols"], dtype=np.int64)
    vals = np.asarray(inputs["vals"], dtype=np.float32)
    x = np.asarray(inputs["x"], dtype=np.float32)
    weight = np.asarray(inputs["weight"], dtype=np.float32)
    bias = np.asarray(inputs["bias"], dtype=np.float32)

    NB, rowsets = _bin_rows(rows, cols)
    percore, TAP, TPBL, TPBH = _split_cells(rows, cols, NB, rowsets)
    ap_calls, ap_tiles, total_ap = _ap_schedule(TAP)

    xi, slabs, w4, wT, iota2d, ident16 = _host_arrays(x, weight)
    maps = []
    for c in range(N_CORES):
        m = _pack_core(c, percore, TAP, TPBL, TPBH, ap_tiles, total_ap,
                       rows, cols, vals, bias, rowsets)
        m.update({"xi": xi, "slabs": slabs, "w4": w4, "wT": wT,
                  "iota": iota2d, "ident16": ident16})
        maps.append(m)
    meta = (NB, TAP, TPBL, TPBH, ap_calls, ap_tiles, total_ap)
    return maps, meta, rowsets, NB


def kernel(x, rows, cols, vals, weight, bias):
    global LAST_RESULTS
    from concourse.bass_utils import run_bass_kernel_spmd

    inputs = {"x": x, "rows": rows, "cols": cols, "vals": vals,
              "weight": weight, "bias": bias}
    maps, meta, rowsets, NB = _prepare(inputs)
    NB_, TAP, TPBL, TPBH, ap_calls, ap_tiles, total_ap = meta

    key = (NB, TAP.tobytes(), TPBL.tobytes(), TPBH.tobytes())
    if key not in _CACHE:
        _CACHE.clear()
        _CACHE[key] = _build(NB, TAP, TPBL, TPBH, ap_calls, ap_tiles, total_ap)
    nc = _CACHE[key]

    res = run_bass_kernel_spmd(nc, maps, core_ids=list(range(N_CORES)))
    LAST_RESULTS = res

    out = np.empty((B, N_OUT, OUT_C), dtype=np.float32)
    for c in range(N_CORES):
        oc = res.results[c]["out"]
        for s in range(NB):
            rowlist = rowsets[c][s]
            if rowlist is None or len(rowlist) == 0:
                continue
            out[:, rowlist, :] = oc[:, s * PB : s * PB + len(rowlist), :]
    return out


def time_hw(inputs, reps=(1, 2049), trials=3):
    import time as _time
    from concourse.bass_utils import run_bass_kernel_spmd

    maps, meta, rowsets, NB = _prepare(inputs)
    NB_, TAP, TPBL, TPBH, ap_calls, ap_tiles, total_ap = meta

    best = {}
    for r in reps:
        nc = _build(NB, TAP, TPBL, TPBH, ap_calls, ap_tiles, total_ap, reps=r)
        run_bass_kernel_spmd(nc, maps, core_ids=list(range(N_CORES)))
        ts = []
        for _ in range(trials):
            t0 = _time.perf_counter()
            run_bass_kernel_spmd(nc, maps, core_ids=list(range(N_CORES)))
            ts.append(_time.perf_counter() - t0)
        best[r] = min(ts)
        print(f"reps={r}: calls {[f'{t*1e3:.1f}ms' for t in ts]}")
    r1, r2 = min(reps), max(reps)
    return (best[r2] - best[r1]) / (r2 - r1) * 1e9
